# revision 13
# baseline (speedup 1.0000x reference)
"""Trainium2 Bass kernel for the GWNN2 GNN (4-graph GraphConv x2 + MLP).

Strategy (8 NeuronCores, dst-sharded):
  * nodes sharded 6250/core (padded 6272); edge lists bucketed host-side by
    (dst-window of 64 nodes, src table half) and padded to fixed chunk counts
  * phase 1: every core computes the full layer-1 projection table
    T1[n, g*128:..] = deg_out_g[n] * (x @ W1)[n]  (bf16, replicated compute)
  * phase 2-4 fused per dst window: SpMM1 via dma_gather of src rows +
    one-hot selection matmul (PSUM accumulate), then l1, l2, W2 projection,
    transpose, deg scale -> own T2 shard
  * AllGather T2 shards -> full T2 table
  * phase 5-6: SpMM2 from T2 + l3 head -> output shard
  * edge weights folded with deg_in^-0.5 host-side; deg_out applied on-chip

The kernel is compiled per call (edge bucket sizes are data-dependent
compile-time constants, identical across cores so one SPMD NEFF serves all 8).
"""
import sys
import types
from dataclasses import dataclass

if "/opt/trn_rl_repo" not in sys.path:
    sys.path.insert(0, "/opt/trn_rl_repo")

import numpy as np
import ml_dtypes

import concourse.bass as bass
import concourse.bacc as bacc
import concourse.mybir as mybir
import concourse.tile as tile
from concourse.masks import make_identity

BF16 = ml_dtypes.bfloat16
P = 128


def _install_ntff_hook():
    """Make trace=True usable under axon (antenv.axon_hooks may be absent)."""
    try:
        import antenv
        if "antenv.axon_hooks" in sys.modules:
            return
        m = types.ModuleType("antenv.axon_hooks")
        box = [None]
        m.set_axon_ntff_profile_hook = lambda h: box.__setitem__(0, h)
        m.get_axon_ntff_profile_hook = lambda: box[0]
        sys.modules["antenv.axon_hooks"] = m
        antenv.axon_hooks = m
        try:
            from trn_agent_boot.trn_boot import _ntff_profile_via_ctypes
            hook = _ntff_profile_via_ctypes("/opt/axon/libaxon_pjrt.so")
            if hook is not None:
                m.set_axon_ntff_profile_hook(hook)
        except Exception:
            pass
    except Exception:
        pass


@dataclass
class Cfg:
    n_nodes: int = 50000
    g_num: int = 4
    in_feats: int = 256
    h_feats: int = 128          # table row width per graph (must be 128)
    n_classes: int = 40
    n_cores: int = 8
    win: int = 128              # dst nodes per SpMM window
    win_batch: int = 4          # windows per dma_gather batch

    @property
    def shard(self):
        return self.n_nodes // self.n_cores

    @property
    def shard_p(self):          # padded shard rows
        return ((self.shard + P - 1) // P) * P

    @property
    def rows(self):             # padded table rows
        return self.shard_p * self.n_cores

    @property
    def half(self):
        return self.rows // 2

    @property
    def nwin(self):
        return self.shard_p // self.win

    @property
    def cat(self):
        return self.h_feats * self.g_num

    @property
    def kc_cat(self):           # 128-chunks in cat dim
        return self.cat // P

    @property
    def kc_in(self):
        return self.in_feats // P

    @property
    def ntile(self):            # node tiles (128) in full padded table
        return self.rows // P

    @property
    def ntile_own(self):
        return self.shard_p // P


def _prep_inputs(cfg: Cfg, in_feat, src, dst, w, W1, W2, l1w, l1b, l2w, l2b,
                 l3w, l3b):
    """Host-side sharding/packing. Returns (in_maps, K_LO, K_HI)."""
    N, G = cfg.n_nodes, cfg.g_num
    SH, SHP = cfg.shard, cfg.shard_p
    NW, WIN = cfg.nwin, cfg.win
    HALF = cfg.half
    src = np.asarray(src).astype(np.int64)
    dst = np.asarray(dst).astype(np.int64)
    w = np.asarray(w, dtype=np.float32)
    in_feat = np.asarray(in_feat, dtype=np.float32)

    deg_out = np.empty((G, N), np.float32)
    deg_in = np.empty((G, N), np.float32)
    for g in range(G):
        deg_out[g] = np.clip(np.bincount(src[g], minlength=N), 1.0, None) ** -0.5
        deg_in[g] = np.clip(np.bincount(dst[g], minlength=N), 1.0, None) ** -0.5

    src_pad = (src // SH) * SHP + (src % SH)          # padded table row
    half_flag = (src_pad >= HALF).astype(np.int64)
    idx_local = (src_pad - half_flag * HALF).astype(np.int64)

    core_of = dst // SH
    dst_loc = dst % SH
    win_of = dst_loc // WIN
    dst_in_win = (dst_loc % WIN).astype(np.float32)

    # first pass: counts to fix K_LO / K_HI globally
    maxlo = maxhi = 1
    buckets = {}
    for i in range(cfg.n_cores):
        for g in range(G):
            m = core_of[g] == i
            key = win_of[g][m] * 2 + half_flag[g][m]
            cnt = np.bincount(key, minlength=NW * 2)
            maxlo = max(maxlo, int(cnt[0::2].max()))
            maxhi = max(maxhi, int(cnt[1::2].max()))
            buckets[(i, g)] = m
    K_LO = (maxlo + P - 1) // P
    K_HI = (maxhi + P - 1) // P

    w_eff = np.empty((G, src.shape[1]), np.float32)
    for g in range(G):
        w_eff[g] = w[g] * deg_in[g][dst[g]]

    # replicated phase-1 inputs (identical for every core)
    xpad = np.zeros((cfg.rows, cfg.in_feats), np.float32)
    for i in range(cfg.n_cores):
        xpad[i * SHP:i * SHP + SH] = in_feat[i * SH:(i + 1) * SH]
    xt4 = xpad.reshape(cfg.ntile, P, cfg.kc_in, P)     # (t, n, kc, k)
    xtiles = np.ascontiguousarray(xt4.transpose(0, 3, 2, 1)).reshape(
        cfg.ntile, P, cfg.kc_in * P).astype(BF16)
    degq = np.zeros((cfg.ntile, P, G), np.float32)
    for g in range(G):
        dp = np.zeros(cfg.rows, np.float32)
        for i in range(cfg.n_cores):
            dp[i * SHP:i * SHP + SH] = deg_out[g, i * SH:(i + 1) * SH]
        degq[:, :, g] = dp.reshape(cfg.ntile, P)

    def pack_lhsT(W, kc):
        Wr = np.asarray(W, np.float32).reshape(kc, P, -1)   # (kc, k, fout)
        return np.ascontiguousarray(Wr.transpose(1, 0, 2)).reshape(P, -1)

    W1c = pack_lhsT(W1, cfg.kc_in).astype(BF16)
    W2c = pack_lhsT(W2, cfg.kc_cat).astype(BF16)
    l1wc = pack_lhsT(l1w, cfg.kc_cat).astype(BF16)
    l2wc = pack_lhsT(l2w, cfg.kc_cat).astype(BF16)
    l3wc = pack_lhsT(l3w, cfg.kc_cat).astype(BF16)
    l1bc = np.ascontiguousarray(
        np.asarray(l1b, np.float32).reshape(cfg.kc_cat, P).T)      # [128, kc]
    l2bc = np.ascontiguousarray(
        np.asarray(l2b, np.float32).reshape(cfg.kc_cat, P).T)
    l3bb = np.tile(np.asarray(l3b, np.float32)[None, :], (P, 1))   # [128, C]

    in_maps = []
    for i in range(cfg.n_cores):
        idx16 = {0: np.zeros((G, NW * K_LO * P), np.int16),
                 1: np.zeros((G, NW * K_HI * P), np.int16)}
        mdst = {0: np.zeros((G, P, NW * K_LO), np.float32),
                1: np.zeros((G, P, NW * K_HI), np.float32)}
        mw = {0: np.zeros((G, P, NW * K_LO), np.float32),
              1: np.zeros((G, P, NW * K_HI), np.float32)}
        for g in range(G):
            m = buckets[(i, g)]
            key = win_of[g][m] * 2 + half_flag[g][m]
            order = np.argsort(key, kind="stable")
            skey = key[order]
            cnt = np.bincount(skey, minlength=NW * 2)
            starts = np.concatenate([[0], np.cumsum(cnt)[:-1]])
            slot = np.arange(len(skey)) - starts[skey]
            il = idx_local[g][m][order]
            dw = dst_in_win[g][m][order]
            we = w_eff[g][m][order]
            swin = skey // 2
            shf = skey % 2
            for h, K in ((0, K_LO), (1, K_HI)):
                sel = shf == h
                pos = swin[sel] * (K * P) + slot[sel]     # (win, c, p) flat
                idx16[h][g][pos] = il[sel].astype(np.int16)
                c = slot[sel] // P
                p = slot[sel] % P
                mdst[h][g][p, swin[sel] * K + c] = dw[sel]
                mw[h][g][p, swin[sel] * K + c] = we[sel]

        GCH = 8                       # chunks (of 128 idx) per dma_gather
        def wrap(arr, K):
            out = np.zeros((G, P, NW * K * 8), np.int16)
            nb = (NW + cfg.win_batch - 1) // cfg.win_batch
            for g in range(G):
                for b in range(nb):
                    w0 = b * cfg.win_batch
                    w1 = min(NW, w0 + cfg.win_batch)
                    nch = (w1 - w0) * K
                    for j in range(0, nch, GCH):
                        gl = min(GCH, nch - j)
                        fl = arr[g][(w0 * K + j) * P: (w0 * K + j + gl) * P]
                        wr = fl.reshape(-1, 16).T          # [16, n]
                        out[g][:, (w0 * K + j) * 8: (w0 * K + j + gl) * 8] =                             np.tile(wr, (8, 1))
            return out

        NTO = cfg.ntile_own
        im = {
            "xtiles": np.ascontiguousarray(xtiles[i * NTO:(i + 1) * NTO]),
            "degq": np.ascontiguousarray(degq[i * NTO:(i + 1) * NTO]),
            "w1c": W1c, "w2c": W2c, "l1wc": l1wc, "l2wc": l2wc,
            "l3wc": l3wc, "l1bc": l1bc, "l2bc": l2bc, "l3bb": l3bb,
            "idx_lo": wrap(idx16[0], K_LO), "idx_hi": wrap(idx16[1], K_HI),
            "mdst_lo": mdst[0].astype(BF16), "mdst_hi": mdst[1].astype(BF16),
            "mw_lo": mw[0].astype(BF16), "mw_hi": mw[1].astype(BF16),
        }
        # own-shard deg_out for phase 4: [WIN, nwin*G], window-major
        degown = np.zeros((WIN, NW * G), np.float32)
        for g in range(G):
            dp = np.zeros(SHP, np.float32)
            dp[:SH] = deg_out[g, i * SH:(i + 1) * SH]
            degown[:, g::G] = dp.reshape(NW, WIN).T
        im["degown"] = degown
        in_maps.append(im)
    return in_maps, K_LO, K_HI


def _build(cfg: Cfg, K_LO, K_HI):
    G, NW, WIN, WB = cfg.g_num, cfg.nwin, cfg.win, cfg.win_batch
    KC = cfg.kc_cat
    HF = cfg.h_feats
    CW = cfg.cat                 # table row width
    CLS = cfg.n_classes
    f32, bf16, i16, i32 = (mybir.dt.float32, mybir.dt.bfloat16,
                           mybir.dt.int16, mybir.dt.int32)

    nc = bacc.Bacc(num_swdge_queues=4)
    t_xt = nc.declare_dram_parameter("xtiles", [cfg.ntile_own, P, cfg.kc_in * P], bf16, isOutput=False)
    t_degq = nc.declare_dram_parameter("degq", [cfg.ntile_own, P, G], f32, isOutput=False)
    t_w1 = nc.declare_dram_parameter("w1c", [P, cfg.kc_in * HF], bf16, isOutput=False)
    t_w2 = nc.declare_dram_parameter("w2c", [P, KC * HF], bf16, isOutput=False)
    t_l1w = nc.declare_dram_parameter("l1wc", [P, KC * CW], bf16, isOutput=False)
    t_l2w = nc.declare_dram_parameter("l2wc", [P, KC * CW], bf16, isOutput=False)
    t_l3w = nc.declare_dram_parameter("l3wc", [P, KC * CLS], bf16, isOutput=False)
    t_l1b = nc.declare_dram_parameter("l1bc", [P, KC], f32, isOutput=False)
    t_l2b = nc.declare_dram_parameter("l2bc", [P, KC], f32, isOutput=False)
    t_l3b = nc.declare_dram_parameter("l3bb", [P, CLS], f32, isOutput=False)
    t_ilo = nc.declare_dram_parameter("idx_lo", [G, P, NW * K_LO * 8], i16, isOutput=False)
    t_ihi = nc.declare_dram_parameter("idx_hi", [G, P, NW * K_HI * 8], i16, isOutput=False)
    t_mdl = nc.declare_dram_parameter("mdst_lo", [G, P, NW * K_LO], bf16, isOutput=False)
    t_mdh = nc.declare_dram_parameter("mdst_hi", [G, P, NW * K_HI], bf16, isOutput=False)
    t_mwl = nc.declare_dram_parameter("mw_lo", [G, P, NW * K_LO], bf16, isOutput=False)
    t_mwh = nc.declare_dram_parameter("mw_hi", [G, P, NW * K_HI], bf16, isOutput=False)
    t_dgo = nc.declare_dram_parameter("degown", [WIN, NW * G], f32, isOutput=False)
    t_out = nc.declare_dram_parameter("out", [WIN, NW * CLS], f32, isOutput=True)

    d_t1s = nc.dram_tensor("t1s", [cfg.shard_p, CW], bf16)
    d_t1f = nc.dram_tensor("t1f", [cfg.rows, CW], bf16, addr_space="Shared")
    d_t2s = nc.dram_tensor("t2s", [cfg.shard_p, CW], bf16)
    d_t2f = nc.dram_tensor("t2f", [cfg.rows, CW], bf16, addr_space="Shared")

    AF = mybir.ActivationFunctionType
    nb = (NW + WB - 1) // WB
    qctr = [0]

    with tile.TileContext(nc) as tc:
        with (
            tc.tile_pool(name="const", bufs=1) as cp,
            tc.tile_pool(name="x", bufs=3) as xp,
            tc.tile_pool(name="gath", bufs=2) as gp,
            tc.tile_pool(name="hcat", bufs=2) as hp,
            tc.tile_pool(name="dense", bufs=3) as dp,
            tc.tile_pool(name="psa", bufs=2, space="PSUM") as pm,
            tc.tile_pool(name="psb", bufs=2, space="PSUM") as pb,
        ):
            # constants
            ident = cp.tile([P, P], f32)
            make_identity(nc, ident[:])
            iota_i = cp.tile([P, WIN], i32)
            nc.gpsimd.iota(iota_i[:], pattern=[[1, WIN]], base=0, channel_multiplier=0)
            iota_b = cp.tile([P, WIN], bf16)
            nc.vector.tensor_copy(iota_b[:], iota_i[:])

            def const_load(t, shape, dtype):
                s = cp.tile(shape, dtype, tag=t.name + "_c")
                nc.sync.dma_start(out=s[:], in_=t[:])
                return s

            w1_sb = const_load(t_w1, [P, cfg.kc_in * HF], bf16)
            w2_sb = const_load(t_w2, [P, KC * HF], bf16)
            l1w_sb = const_load(t_l1w, [P, KC * CW], bf16)
            l2w_sb = const_load(t_l2w, [P, KC * CW], bf16)
            l3w_sb = const_load(t_l3w, [P, KC * CLS], bf16)
            l1b_sb = const_load(t_l1b, [P, KC], f32)
            l2b_sb = const_load(t_l2b, [P, KC], f32)
            l3b_sb = const_load(t_l3b, [P, CLS], f32)
            dgo_sb = const_load(t_dgo, [WIN, NW * G], f32)
            out_sb = cp.tile([WIN, NW * CLS], f32)

            # ---------------- phase 1: replicated T1 ----------------
            for t in range(cfg.ntile_own):
                xt = xp.tile([P, cfg.kc_in * P], bf16, tag="xt")
                nc.sync.dma_start(out=xt[:], in_=t_xt[t])
                dq = xp.tile([P, G], f32, tag="dq")
                nc.sync.dma_start(out=dq[:], in_=t_degq[t])
                q1 = pb.tile([P, HF], f32, tag="misc")
                for kc in range(cfg.kc_in):
                    nc.tensor.matmul(
                        out=q1[:], lhsT=xt[:, kc * P:(kc + 1) * P],
                        rhs=w1_sb[:, kc * HF:(kc + 1) * HF],
                        start=(kc == 0), stop=(kc == cfg.kc_in - 1))
                h1row = xp.tile([P, CW], bf16, tag="h1row")
                for g in range(G):
                    nc.scalar.activation(h1row[:, g * HF:(g + 1) * HF], q1[:],
                                         AF.Copy, scale=dq[:, g:g + 1])
                nc.sync.dma_start(out=d_t1s[t * P:(t + 1) * P, :], in_=h1row[:])

            tc.strict_bb_all_engine_barrier()
            nc.gpsimd.collective_compute(
                "AllGather", mybir.AluOpType.bypass,
                ins=[d_t1s[:]], outs=[d_t1f[:]],
                replica_groups=[list(range(cfg.n_cores))],
            )

            # ------------- SpMM + dense layers, per window batch -------------
            def spmm_layer(table, layer2):
                for b in range(nb):
                    w0 = b * WB
                    w1 = min(NW, w0 + WB)
                    nw = w1 - w0
                    hcat_t = {}
                    for g in range(G):
                        feats = {}
                        sels = {}
                        for h, K, t_i, t_md, t_mw in (
                            (0, K_LO, t_ilo, t_mdl, t_mwl),
                            (1, K_HI, t_ihi, t_mdh, t_mwh),
                        ):
                            ni = nw * K * P
                            idx_t = gp.tile([P, WB * K * 8], i16, tag=f"idx{h}")
                            nc.sync.dma_start(
                                out=idx_t[:, :nw * K * 8],
                                in_=t_i[g][:, w0 * K * 8:w1 * K * 8])
                            md_t = gp.tile([P, WB * K], bf16, tag=f"md{h}")
                            nc.sync.dma_start(out=md_t[:, :nw * K],
                                              in_=t_md[g][:, w0 * K:w1 * K])
                            mw_t = gp.tile([P, WB * K], bf16, tag=f"mw{h}")
                            nc.sync.dma_start(out=mw_t[:, :nw * K],
                                              in_=t_mw[g][:, w0 * K:w1 * K])
                            ft = gp.tile([P, WB * K * HF], bf16, tag=f"ft{h}")
                            GCH = 8
                            nch = nw * K
                            for j in range(0, nch, GCH):
                                gl = min(GCH, nch - j)
                                nij = gl * P
                                nc.gpsimd.dma_gather(
                                    out_ap=ft[:, j * HF:(j + gl) * HF].rearrange(
                                        "p (k f) -> p k f", f=HF),
                                    in_ap=table[(cfg.half if h else 0):
                                                (cfg.rows if h else cfg.half),
                                                g * HF:(g + 1) * HF],
                                    idxs_ap=idx_t[:, j * 8:(j + gl) * 8],
                                    num_idxs=nij, num_idxs_reg=nij,
                                    elem_size=HF, elem_step=CW,
                                    queue_num=qctr[0] % 4,
                                )
                                qctr[0] += 1
                            st = gp.tile([P, WB * K * WIN], bf16, tag=f"st{h}")
                            s3 = st[:, :nw * K * WIN].rearrange(
                                "p (k x) -> p k x", x=WIN)
                            ib = iota_b[:]
                            nc.vector.tensor_tensor(
                                out=s3,
                                in0=md_t[:, :nw * K].to_broadcast([P, nw * K, WIN]),
                                in1=bass.AP(ib.tensor, ib.offset,
                                            [list(ib.ap[0]), [0, nw * K],
                                             list(ib.ap[1])]),
                                op=mybir.AluOpType.is_equal)
                            nc.vector.tensor_tensor(
                                out=s3, in0=s3,
                                in1=mw_t[:, :nw * K].to_broadcast([P, nw * K, WIN]),
                                op=mybir.AluOpType.mult)
                            feats[h] = ft
                            sels[h] = st
                        for wi in range(w0, w1):
                            dw = wi - w0
                            ps = pm.tile([P, WIN], f32, tag="agg")
                            tot = K_LO + K_HI
                            ci = 0
                            for h, K in ((0, K_LO), (1, K_HI)):
                                ft, st = feats[h], sels[h]
                                for c in range(K):
                                    cc = dw * K + c
                                    nc.tensor.matmul(
                                        out=ps[:],
                                        lhsT=ft[:, cc * HF:(cc + 1) * HF],
                                        rhs=st[:, cc * WIN:(cc + 1) * WIN],
                                        start=(ci == 0), stop=(ci == tot - 1))
                                    ci += 1
                            hc = hp.tile([P, WIN], bf16, tag=f"hc{dw}_{g}")
                            nc.scalar.activation(hc[:], ps[:], AF.Relu)
                            hcat_t[(wi, g)] = hc
                    for wi in range(w0, w1):
                        hcat = [hcat_t[(wi, g)] for g in range(G)]
                        if not layer2:
                            def mlp(ws, bs, ins, name):
                                outs = []
                                for fc in range(KC):
                                    ps = pm.tile([P, WIN], f32, tag="mlp")
                                    for kc in range(KC):
                                        nc.tensor.matmul(
                                            out=ps[:],
                                            lhsT=ws[:, (kc * KC + fc) * P:
                                                    (kc * KC + fc + 1) * P],
                                            rhs=ins[kc][:],
                                            start=(kc == 0), stop=(kc == KC - 1))
                                    o = dp.tile([P, WIN], bf16,
                                                tag=f"mlpo{name}{fc}")
                                    nc.scalar.activation(o[:], ps[:], AF.Relu,
                                                         bias=bs[:, fc:fc + 1])
                                    outs.append(o)
                                return outs
                            hl1 = mlp(l1w_sb, l1b_sb, hcat, "a")
                            hl2 = mlp(l2w_sb, l2b_sb, hl1, "b")
                            p2 = pb.tile([P, WIN], f32, tag="misc")
                            for kc in range(KC):
                                nc.tensor.matmul(
                                    out=p2[:],
                                    lhsT=w2_sb[:, kc * HF:(kc + 1) * HF],
                                    rhs=hl2[kc][:],
                                    start=(kc == 0), stop=(kc == KC - 1))
                            p2s = dp.tile([P, WIN], f32, tag="p2s")
                            nc.scalar.activation(p2s[:], p2[:], AF.Copy)
                            p2t = pb.tile([WIN, P], f32, tag="misc")
                            nc.tensor.transpose(p2t[:], p2s[:], ident[:])
                            h2r = dp.tile([WIN, CW], bf16, tag="h2r")
                            for g in range(G):
                                nc.scalar.activation(
                                    h2r[:, g * HF:(g + 1) * HF], p2t[:], AF.Copy,
                                    scale=dgo_sb[:, wi * G + g:wi * G + g + 1])
                            nc.sync.dma_start(
                                out=d_t2s[wi * WIN:(wi + 1) * WIN, :], in_=h2r[:])
                        else:
                            ps = pb.tile([WIN, CLS], f32, tag="misc")
                            for kc in range(KC):
                                nc.tensor.matmul(
                                    out=ps[:],
                                    lhsT=hcat[kc][:],
                                    rhs=l3w_sb[:, kc * CLS:(kc + 1) * CLS],
                                    start=(kc == 0), stop=(kc == KC - 1))
                            nc.vector.tensor_tensor(
                                out=out_sb[:, wi * CLS:(wi + 1) * CLS],
                                in0=ps[:], in1=l3b_sb[:WIN, :],
                                op=mybir.AluOpType.add)

            spmm_layer(d_t1f, layer2=False)

            tc.strict_bb_all_engine_barrier()
            nc.gpsimd.collective_compute(
                "AllGather", mybir.AluOpType.bypass,
                ins=[d_t2s[:]], outs=[d_t2f[:]],
                replica_groups=[list(range(cfg.n_cores))],
            )

            spmm_layer(d_t2f, layer2=True)

            nc.sync.dma_start(out=t_out[:], in_=out_sb[:])
    nc.finalize()
    return nc


def _run(cfg: Cfg, inputs: dict, trace: bool = False):
    _install_ntff_hook()
    from concourse import bass_utils
    bass_utils.upload_artifacts = lambda d: "local://skipped"
    from concourse.bass_utils import run_bass_kernel_spmd

    in_maps, K_LO, K_HI = _prep_inputs(cfg, **inputs)
    nc = _build(cfg, K_LO, K_HI)
    res = run_bass_kernel_spmd(nc, in_maps, list(range(cfg.n_cores)),
                               trace=trace)
    outs = []
    for i in range(cfg.n_cores):
        o = res.results[i]["out"]                     # [WIN, nwin*CLS]
        o = o.reshape(cfg.win, cfg.nwin, cfg.n_classes).transpose(1, 0, 2)
        outs.append(o.reshape(cfg.shard_p, cfg.n_classes)[:cfg.shard])
    full = np.concatenate(outs, axis=0)
    return full, res.exec_time_ns


def kernel(**inputs) -> np.ndarray:
    cfg = Cfg()
    out, _ = _run(cfg, inputs, trace=False)
    return out.astype(np.float32)


# revision 14
# speedup vs baseline: 1.0063x; 1.0063x over previous
"""Trainium2 Bass kernel for the GWNN2 GNN (4-graph GraphConv x2 + MLP).

Strategy (8 NeuronCores, dst-sharded):
  * nodes sharded 6250/core (padded 6272); edge lists bucketed host-side by
    (dst-window of 64 nodes, src table half) and padded to fixed chunk counts
  * phase 1: every core computes the full layer-1 projection table
    T1[n, g*128:..] = deg_out_g[n] * (x @ W1)[n]  (bf16, replicated compute)
  * phase 2-4 fused per dst window: SpMM1 via dma_gather of src rows +
    one-hot selection matmul (PSUM accumulate), then l1, l2, W2 projection,
    transpose, deg scale -> own T2 shard
  * AllGather T2 shards -> full T2 table
  * phase 5-6: SpMM2 from T2 + l3 head -> output shard
  * edge weights folded with deg_in^-0.5 host-side; deg_out applied on-chip

The kernel is compiled per call (edge bucket sizes are data-dependent
compile-time constants, identical across cores so one SPMD NEFF serves all 8).
"""
import sys
import types
from dataclasses import dataclass

if "/opt/trn_rl_repo" not in sys.path:
    sys.path.insert(0, "/opt/trn_rl_repo")

import numpy as np
import ml_dtypes

import concourse.bass as bass
import concourse.bacc as bacc
import concourse.mybir as mybir
import concourse.tile as tile
from concourse.masks import make_identity

BF16 = ml_dtypes.bfloat16
P = 128


def _install_ntff_hook():
    """Make trace=True usable under axon (antenv.axon_hooks may be absent)."""
    try:
        import antenv
        if "antenv.axon_hooks" in sys.modules:
            return
        m = types.ModuleType("antenv.axon_hooks")
        box = [None]
        m.set_axon_ntff_profile_hook = lambda h: box.__setitem__(0, h)
        m.get_axon_ntff_profile_hook = lambda: box[0]
        sys.modules["antenv.axon_hooks"] = m
        antenv.axon_hooks = m
        try:
            from trn_agent_boot.trn_boot import _ntff_profile_via_ctypes
            hook = _ntff_profile_via_ctypes("/opt/axon/libaxon_pjrt.so")
            if hook is not None:
                m.set_axon_ntff_profile_hook(hook)
        except Exception:
            pass
    except Exception:
        pass


@dataclass
class Cfg:
    n_nodes: int = 50000
    g_num: int = 4
    in_feats: int = 256
    h_feats: int = 128          # table row width per graph (must be 128)
    n_classes: int = 40
    n_cores: int = 8
    win: int = 128              # dst nodes per SpMM window
    win_batch: int = 4          # windows per dma_gather batch

    @property
    def shard(self):
        return self.n_nodes // self.n_cores

    @property
    def shard_p(self):          # padded shard rows
        return ((self.shard + P - 1) // P) * P

    @property
    def rows(self):             # padded table rows
        return self.shard_p * self.n_cores

    @property
    def half(self):
        return self.rows // 2

    @property
    def nwin(self):
        return self.shard_p // self.win

    @property
    def cat(self):
        return self.h_feats * self.g_num

    @property
    def kc_cat(self):           # 128-chunks in cat dim
        return self.cat // P

    @property
    def kc_in(self):
        return self.in_feats // P

    @property
    def ntile(self):            # node tiles (128) in full padded table
        return self.rows // P

    @property
    def ntile_own(self):
        return self.shard_p // P


def _prep_inputs(cfg: Cfg, in_feat, src, dst, w, W1, W2, l1w, l1b, l2w, l2b,
                 l3w, l3b):
    """Host-side sharding/packing. Returns (in_maps, K_LO, K_HI)."""
    N, G = cfg.n_nodes, cfg.g_num
    SH, SHP = cfg.shard, cfg.shard_p
    NW, WIN = cfg.nwin, cfg.win
    HALF = cfg.half
    src = np.asarray(src).astype(np.int64)
    dst = np.asarray(dst).astype(np.int64)
    w = np.asarray(w, dtype=np.float32)
    in_feat = np.asarray(in_feat, dtype=np.float32)

    deg_out = np.empty((G, N), np.float32)
    deg_in = np.empty((G, N), np.float32)
    for g in range(G):
        deg_out[g] = np.clip(np.bincount(src[g], minlength=N), 1.0, None) ** -0.5
        deg_in[g] = np.clip(np.bincount(dst[g], minlength=N), 1.0, None) ** -0.5

    src_pad = (src // SH) * SHP + (src % SH)          # padded table row
    half_flag = (src_pad >= HALF).astype(np.int64)
    idx_local = (src_pad - half_flag * HALF).astype(np.int64)

    core_of = dst // SH
    dst_loc = dst % SH
    win_of = dst_loc // WIN
    dst_in_win = (dst_loc % WIN).astype(np.float32)

    # first pass: counts to fix K_LO / K_HI globally
    maxlo = maxhi = 1
    buckets = {}
    for i in range(cfg.n_cores):
        for g in range(G):
            m = core_of[g] == i
            key = win_of[g][m] * 2 + half_flag[g][m]
            cnt = np.bincount(key, minlength=NW * 2)
            maxlo = max(maxlo, int(cnt[0::2].max()))
            maxhi = max(maxhi, int(cnt[1::2].max()))
            buckets[(i, g)] = m
    K_LO = (maxlo + P - 1) // P
    K_HI = (maxhi + P - 1) // P

    w_eff = np.empty((G, src.shape[1]), np.float32)
    for g in range(G):
        w_eff[g] = w[g] * deg_in[g][dst[g]]

    # replicated phase-1 inputs (identical for every core)
    xpad = np.zeros((cfg.rows, cfg.in_feats), np.float32)
    for i in range(cfg.n_cores):
        xpad[i * SHP:i * SHP + SH] = in_feat[i * SH:(i + 1) * SH]
    xt4 = xpad.reshape(cfg.ntile, P, cfg.kc_in, P)     # (t, n, kc, k)
    xtiles = np.ascontiguousarray(xt4.transpose(0, 3, 2, 1)).reshape(
        cfg.ntile, P, cfg.kc_in * P).astype(BF16)
    degq = np.zeros((cfg.ntile, P, G), np.float32)
    for g in range(G):
        dp = np.zeros(cfg.rows, np.float32)
        for i in range(cfg.n_cores):
            dp[i * SHP:i * SHP + SH] = deg_out[g, i * SH:(i + 1) * SH]
        degq[:, :, g] = dp.reshape(cfg.ntile, P)

    def pack_lhsT(W, kc):
        Wr = np.asarray(W, np.float32).reshape(kc, P, -1)   # (kc, k, fout)
        return np.ascontiguousarray(Wr.transpose(1, 0, 2)).reshape(P, -1)

    W1c = pack_lhsT(W1, cfg.kc_in).astype(BF16)
    W2c = pack_lhsT(W2, cfg.kc_cat).astype(BF16)
    l1wc = pack_lhsT(l1w, cfg.kc_cat).astype(BF16)
    l2wc = pack_lhsT(l2w, cfg.kc_cat).astype(BF16)
    l3wc = pack_lhsT(l3w, cfg.kc_cat).astype(BF16)
    l1bc = np.ascontiguousarray(
        np.asarray(l1b, np.float32).reshape(cfg.kc_cat, P).T)      # [128, kc]
    l2bc = np.ascontiguousarray(
        np.asarray(l2b, np.float32).reshape(cfg.kc_cat, P).T)
    l3bb = np.tile(np.asarray(l3b, np.float32)[None, :], (P, 1))   # [128, C]

    in_maps = []
    for i in range(cfg.n_cores):
        idx16 = {0: np.zeros((G, NW * K_LO * P), np.int16),
                 1: np.zeros((G, NW * K_HI * P), np.int16)}
        mdst = {0: np.zeros((G, P, NW * K_LO), np.float32),
                1: np.zeros((G, P, NW * K_HI), np.float32)}
        mw = {0: np.zeros((G, P, NW * K_LO), np.float32),
              1: np.zeros((G, P, NW * K_HI), np.float32)}
        for g in range(G):
            m = buckets[(i, g)]
            key = win_of[g][m] * 2 + half_flag[g][m]
            order = np.argsort(key, kind="stable")
            skey = key[order]
            cnt = np.bincount(skey, minlength=NW * 2)
            starts = np.concatenate([[0], np.cumsum(cnt)[:-1]])
            slot = np.arange(len(skey)) - starts[skey]
            il = idx_local[g][m][order]
            dw = dst_in_win[g][m][order]
            we = w_eff[g][m][order]
            swin = skey // 2
            shf = skey % 2
            for h, K in ((0, K_LO), (1, K_HI)):
                sel = shf == h
                pos = swin[sel] * (K * P) + slot[sel]     # (win, c, p) flat
                idx16[h][g][pos] = il[sel].astype(np.int16)
                c = slot[sel] // P
                p = slot[sel] % P
                mdst[h][g][p, swin[sel] * K + c] = dw[sel]
                mw[h][g][p, swin[sel] * K + c] = we[sel]

        GCH = 8                       # chunks (of 128 idx) per dma_gather
        def wrap(arr, K):
            out = np.zeros((G, P, NW * K * 8), np.int16)
            nb = (NW + cfg.win_batch - 1) // cfg.win_batch
            for g in range(G):
                for b in range(nb):
                    w0 = b * cfg.win_batch
                    w1 = min(NW, w0 + cfg.win_batch)
                    nch = (w1 - w0) * K
                    for j in range(0, nch, GCH):
                        gl = min(GCH, nch - j)
                        fl = arr[g][(w0 * K + j) * P: (w0 * K + j + gl) * P]
                        wr = fl.reshape(-1, 16).T          # [16, n]
                        out[g][:, (w0 * K + j) * 8: (w0 * K + j + gl) * 8] =                             np.tile(wr, (8, 1))
            return out

        NTO = cfg.ntile_own
        im = {
            "xtiles": np.ascontiguousarray(xtiles[i * NTO:(i + 1) * NTO]),
            "degq": np.ascontiguousarray(degq[i * NTO:(i + 1) * NTO]),
            "w1c": W1c, "w2c": W2c, "l1wc": l1wc, "l2wc": l2wc,
            "l3wc": l3wc, "l1bc": l1bc, "l2bc": l2bc, "l3bb": l3bb,
            "idx_lo": wrap(idx16[0], K_LO), "idx_hi": wrap(idx16[1], K_HI),
            "mdst_lo": mdst[0].astype(BF16), "mdst_hi": mdst[1].astype(BF16),
            "mw_lo": mw[0].astype(BF16), "mw_hi": mw[1].astype(BF16),
        }
        # own-shard deg_out for phase 4: [WIN, nwin*G], window-major
        degown = np.zeros((WIN, NW * G), np.float32)
        for g in range(G):
            dp = np.zeros(SHP, np.float32)
            dp[:SH] = deg_out[g, i * SH:(i + 1) * SH]
            degown[:, g::G] = dp.reshape(NW, WIN).T
        im["degown"] = degown
        in_maps.append(im)
    return in_maps, K_LO, K_HI


def _build(cfg: Cfg, K_LO, K_HI):
    G, NW, WIN, WB = cfg.g_num, cfg.nwin, cfg.win, cfg.win_batch
    KC = cfg.kc_cat
    HF = cfg.h_feats
    CW = cfg.cat                 # table row width
    CLS = cfg.n_classes
    f32, bf16, i16, i32 = (mybir.dt.float32, mybir.dt.bfloat16,
                           mybir.dt.int16, mybir.dt.int32)

    nc = bacc.Bacc(num_swdge_queues=4)
    t_xt = nc.declare_dram_parameter("xtiles", [cfg.ntile_own, P, cfg.kc_in * P], bf16, isOutput=False)
    t_degq = nc.declare_dram_parameter("degq", [cfg.ntile_own, P, G], f32, isOutput=False)
    t_w1 = nc.declare_dram_parameter("w1c", [P, cfg.kc_in * HF], bf16, isOutput=False)
    t_w2 = nc.declare_dram_parameter("w2c", [P, KC * HF], bf16, isOutput=False)
    t_l1w = nc.declare_dram_parameter("l1wc", [P, KC * CW], bf16, isOutput=False)
    t_l2w = nc.declare_dram_parameter("l2wc", [P, KC * CW], bf16, isOutput=False)
    t_l3w = nc.declare_dram_parameter("l3wc", [P, KC * CLS], bf16, isOutput=False)
    t_l1b = nc.declare_dram_parameter("l1bc", [P, KC], f32, isOutput=False)
    t_l2b = nc.declare_dram_parameter("l2bc", [P, KC], f32, isOutput=False)
    t_l3b = nc.declare_dram_parameter("l3bb", [P, CLS], f32, isOutput=False)
    t_ilo = nc.declare_dram_parameter("idx_lo", [G, P, NW * K_LO * 8], i16, isOutput=False)
    t_ihi = nc.declare_dram_parameter("idx_hi", [G, P, NW * K_HI * 8], i16, isOutput=False)
    t_mdl = nc.declare_dram_parameter("mdst_lo", [G, P, NW * K_LO], bf16, isOutput=False)
    t_mdh = nc.declare_dram_parameter("mdst_hi", [G, P, NW * K_HI], bf16, isOutput=False)
    t_mwl = nc.declare_dram_parameter("mw_lo", [G, P, NW * K_LO], bf16, isOutput=False)
    t_mwh = nc.declare_dram_parameter("mw_hi", [G, P, NW * K_HI], bf16, isOutput=False)
    t_dgo = nc.declare_dram_parameter("degown", [WIN, NW * G], f32, isOutput=False)
    t_out = nc.declare_dram_parameter("out", [WIN, NW * CLS], f32, isOutput=True)

    d_t1s = nc.dram_tensor("t1s", [cfg.shard_p, CW], bf16)
    d_t1f = nc.dram_tensor("t1f", [cfg.rows, CW], bf16, addr_space="Shared")
    d_t2s = nc.dram_tensor("t2s", [cfg.shard_p, CW], bf16)
    d_t2f = nc.dram_tensor("t2f", [cfg.rows, CW], bf16, addr_space="Shared")

    AF = mybir.ActivationFunctionType
    nb = (NW + WB - 1) // WB
    qctr = [0]

    with tile.TileContext(nc) as tc:
        with (
            tc.tile_pool(name="const", bufs=1) as cp,
            tc.tile_pool(name="x", bufs=3) as xp,
            tc.tile_pool(name="gath", bufs=2) as gp,
            tc.tile_pool(name="hcat", bufs=2) as hp,
            tc.tile_pool(name="dense", bufs=3) as dp,
            tc.tile_pool(name="psa", bufs=2, space="PSUM") as pm,
            tc.tile_pool(name="psb", bufs=2, space="PSUM") as pb,
        ):
            # constants
            ident = cp.tile([P, P], f32)
            make_identity(nc, ident[:])
            iota_i = cp.tile([P, WIN], i32)
            nc.gpsimd.iota(iota_i[:], pattern=[[1, WIN]], base=0, channel_multiplier=0)
            iota_b = cp.tile([P, WIN], bf16)
            nc.vector.tensor_copy(iota_b[:], iota_i[:])

            def const_load(t, shape, dtype):
                s = cp.tile(shape, dtype, tag=t.name + "_c")
                nc.sync.dma_start(out=s[:], in_=t[:])
                return s

            w1_sb = const_load(t_w1, [P, cfg.kc_in * HF], bf16)
            w2_sb = const_load(t_w2, [P, KC * HF], bf16)
            l1w_sb = const_load(t_l1w, [P, KC * CW], bf16)
            l2w_sb = const_load(t_l2w, [P, KC * CW], bf16)
            l3w_sb = const_load(t_l3w, [P, KC * CLS], bf16)
            l1b_sb = const_load(t_l1b, [P, KC], f32)
            l2b_sb = const_load(t_l2b, [P, KC], f32)
            l3b_sb = const_load(t_l3b, [P, CLS], f32)
            dgo_sb = const_load(t_dgo, [WIN, NW * G], f32)
            out_sb = cp.tile([WIN, NW * CLS], f32)

            # ---------------- phase 1: replicated T1 ----------------
            for t in range(cfg.ntile_own):
                xt = xp.tile([P, cfg.kc_in * P], bf16, tag="xt")
                nc.sync.dma_start(out=xt[:], in_=t_xt[t])
                dq = xp.tile([P, G], f32, tag="dq")
                nc.sync.dma_start(out=dq[:], in_=t_degq[t])
                q1 = pb.tile([P, HF], f32, tag="misc")
                for kc in range(cfg.kc_in):
                    nc.tensor.matmul(
                        out=q1[:], lhsT=xt[:, kc * P:(kc + 1) * P],
                        rhs=w1_sb[:, kc * HF:(kc + 1) * HF],
                        start=(kc == 0), stop=(kc == cfg.kc_in - 1))
                h1row = xp.tile([P, CW], bf16, tag="h1row")
                for g in range(G):
                    nc.scalar.activation(h1row[:, g * HF:(g + 1) * HF], q1[:],
                                         AF.Copy, scale=dq[:, g:g + 1])
                nc.sync.dma_start(out=d_t1s[t * P:(t + 1) * P, :], in_=h1row[:])

            tc.strict_bb_all_engine_barrier()
            nc.gpsimd.collective_compute(
                "AllGather", mybir.AluOpType.bypass,
                ins=[d_t1s[:]], outs=[d_t1f[:]],
                replica_groups=[list(range(cfg.n_cores))],
            )
            tc.strict_bb_all_engine_barrier()

            # ------------- SpMM + dense layers, per window batch -------------
            def spmm_layer(table, layer2):
                for b in range(nb):
                    w0 = b * WB
                    w1 = min(NW, w0 + WB)
                    nw = w1 - w0
                    hcat_t = {}
                    for g in range(G):
                        feats = {}
                        sels = {}
                        for h, K, t_i, t_md, t_mw in (
                            (0, K_LO, t_ilo, t_mdl, t_mwl),
                            (1, K_HI, t_ihi, t_mdh, t_mwh),
                        ):
                            ni = nw * K * P
                            idx_t = gp.tile([P, WB * K * 8], i16, tag=f"idx{h}")
                            nc.sync.dma_start(
                                out=idx_t[:, :nw * K * 8],
                                in_=t_i[g][:, w0 * K * 8:w1 * K * 8])
                            md_t = gp.tile([P, WB * K], bf16, tag=f"md{h}")
                            nc.sync.dma_start(out=md_t[:, :nw * K],
                                              in_=t_md[g][:, w0 * K:w1 * K])
                            mw_t = gp.tile([P, WB * K], bf16, tag=f"mw{h}")
                            nc.sync.dma_start(out=mw_t[:, :nw * K],
                                              in_=t_mw[g][:, w0 * K:w1 * K])
                            ft = gp.tile([P, WB * K * HF], bf16, tag=f"ft{h}")
                            GCH = 8
                            nch = nw * K
                            for j in range(0, nch, GCH):
                                gl = min(GCH, nch - j)
                                nij = gl * P
                                nc.gpsimd.dma_gather(
                                    out_ap=ft[:, j * HF:(j + gl) * HF].rearrange(
                                        "p (k f) -> p k f", f=HF),
                                    in_ap=table[(cfg.half if h else 0):
                                                (cfg.rows if h else cfg.half),
                                                g * HF:(g + 1) * HF],
                                    idxs_ap=idx_t[:, j * 8:(j + gl) * 8],
                                    num_idxs=nij, num_idxs_reg=nij,
                                    elem_size=HF, elem_step=CW,
                                    queue_num=qctr[0] % 4,
                                )
                                qctr[0] += 1
                            st = gp.tile([P, WB * K * WIN], bf16, tag=f"st{h}")
                            s3 = st[:, :nw * K * WIN].rearrange(
                                "p (k x) -> p k x", x=WIN)
                            ib = iota_b[:]
                            nc.vector.tensor_tensor(
                                out=s3,
                                in0=md_t[:, :nw * K].to_broadcast([P, nw * K, WIN]),
                                in1=bass.AP(ib.tensor, ib.offset,
                                            [list(ib.ap[0]), [0, nw * K],
                                             list(ib.ap[1])]),
                                op=mybir.AluOpType.is_equal)
                            nc.vector.tensor_tensor(
                                out=s3, in0=s3,
                                in1=mw_t[:, :nw * K].to_broadcast([P, nw * K, WIN]),
                                op=mybir.AluOpType.mult)
                            feats[h] = ft
                            sels[h] = st
                        for wi in range(w0, w1):
                            dw = wi - w0
                            ps = pm.tile([P, WIN], f32, tag="agg")
                            tot = K_LO + K_HI
                            ci = 0
                            for h, K in ((0, K_LO), (1, K_HI)):
                                ft, st = feats[h], sels[h]
                                for c in range(K):
                                    cc = dw * K + c
                                    nc.tensor.matmul(
                                        out=ps[:],
                                        lhsT=ft[:, cc * HF:(cc + 1) * HF],
                                        rhs=st[:, cc * WIN:(cc + 1) * WIN],
                                        start=(ci == 0), stop=(ci == tot - 1))
                                    ci += 1
                            hc = hp.tile([P, WIN], bf16, tag=f"hc{dw}_{g}")
                            nc.scalar.activation(hc[:], ps[:], AF.Relu)
                            hcat_t[(wi, g)] = hc
                    for wi in range(w0, w1):
                        hcat = [hcat_t[(wi, g)] for g in range(G)]
                        if not layer2:
                            def mlp(ws, bs, ins, name):
                                outs = []
                                for fc in range(KC):
                                    ps = pm.tile([P, WIN], f32, tag="mlp")
                                    for kc in range(KC):
                                        nc.tensor.matmul(
                                            out=ps[:],
                                            lhsT=ws[:, (kc * KC + fc) * P:
                                                    (kc * KC + fc + 1) * P],
                                            rhs=ins[kc][:],
                                            start=(kc == 0), stop=(kc == KC - 1))
                                    o = dp.tile([P, WIN], bf16,
                                                tag=f"mlpo{name}{fc}")
                                    nc.scalar.activation(o[:], ps[:], AF.Relu,
                                                         bias=bs[:, fc:fc + 1])
                                    outs.append(o)
                                return outs
                            hl1 = mlp(l1w_sb, l1b_sb, hcat, "a")
                            hl2 = mlp(l2w_sb, l2b_sb, hl1, "b")
                            p2 = pb.tile([P, WIN], f32, tag="misc")
                            for kc in range(KC):
                                nc.tensor.matmul(
                                    out=p2[:],
                                    lhsT=w2_sb[:, kc * HF:(kc + 1) * HF],
                                    rhs=hl2[kc][:],
                                    start=(kc == 0), stop=(kc == KC - 1))
                            p2s = dp.tile([P, WIN], f32, tag="p2s")
                            nc.scalar.activation(p2s[:], p2[:], AF.Copy)
                            p2t = pb.tile([WIN, P], f32, tag="misc")
                            nc.tensor.transpose(p2t[:], p2s[:], ident[:])
                            h2r = dp.tile([WIN, CW], bf16, tag="h2r")
                            for g in range(G):
                                nc.scalar.activation(
                                    h2r[:, g * HF:(g + 1) * HF], p2t[:], AF.Copy,
                                    scale=dgo_sb[:, wi * G + g:wi * G + g + 1])
                            nc.sync.dma_start(
                                out=d_t2s[wi * WIN:(wi + 1) * WIN, :], in_=h2r[:])
                        else:
                            ps = pb.tile([WIN, CLS], f32, tag="misc")
                            for kc in range(KC):
                                nc.tensor.matmul(
                                    out=ps[:],
                                    lhsT=hcat[kc][:],
                                    rhs=l3w_sb[:, kc * CLS:(kc + 1) * CLS],
                                    start=(kc == 0), stop=(kc == KC - 1))
                            nc.vector.tensor_tensor(
                                out=out_sb[:, wi * CLS:(wi + 1) * CLS],
                                in0=ps[:], in1=l3b_sb[:WIN, :],
                                op=mybir.AluOpType.add)

            spmm_layer(d_t1f, layer2=False)

            tc.strict_bb_all_engine_barrier()
            nc.gpsimd.collective_compute(
                "AllGather", mybir.AluOpType.bypass,
                ins=[d_t2s[:]], outs=[d_t2f[:]],
                replica_groups=[list(range(cfg.n_cores))],
            )
            tc.strict_bb_all_engine_barrier()

            spmm_layer(d_t2f, layer2=True)

            nc.sync.dma_start(out=t_out[:], in_=out_sb[:])
    nc.finalize()
    return nc


def _run(cfg: Cfg, inputs: dict, trace: bool = False):
    _install_ntff_hook()
    from concourse import bass_utils
    bass_utils.upload_artifacts = lambda d: "local://skipped"
    from concourse.bass_utils import run_bass_kernel_spmd

    in_maps, K_LO, K_HI = _prep_inputs(cfg, **inputs)
    nc = _build(cfg, K_LO, K_HI)
    res = run_bass_kernel_spmd(nc, in_maps, list(range(cfg.n_cores)),
                               trace=trace)
    outs = []
    for i in range(cfg.n_cores):
        o = res.results[i]["out"]                     # [WIN, nwin*CLS]
        o = o.reshape(cfg.win, cfg.nwin, cfg.n_classes).transpose(1, 0, 2)
        outs.append(o.reshape(cfg.shard_p, cfg.n_classes)[:cfg.shard])
    full = np.concatenate(outs, axis=0)
    return full, res.exec_time_ns


def kernel(**inputs) -> np.ndarray:
    cfg = Cfg()
    out, _ = _run(cfg, inputs, trace=False)
    return out.astype(np.float32)
